# revision 23
# baseline (speedup 1.0000x reference)
"""Multi-head self-attention Trainium2 kernel (8 NeuronCores, SPMD).

Problem: x[2,2048,1024] f32, 16 heads x 64 dim, full QKV+attention+output
projection. Sharding: core = (batch n, head-group of 4 heads). Each core
computes partial^T = Wo_rows^T @ head_out^T for its 4 heads; host sums the
4 partials per batch and transposes back.

v5 design (ACT-bound flat pipeline):
  - Heads in pairs (0,1), (2,3). QT/KT bf16 pair tiles [128, S]: head h at
    partitions 0-63, h+1 at 64-127. Scores run as two row-tiled (64x128
    mode) matmuls executing CONCURRENTLY in the PE array.
  - One exp ACT op per (pair, key-chunk) [128, 1024] -> fp8 ex; mask
    (fp8 0/1, pre-duplicated) applied multiplicatively on DVE.
  - PV in fp8 DoubleRow over KEY-CHUNK PAIRS (virtual K=256): half the
    matmuls of bf16, same 216ns each. lhsT = [V_h | 1] (M=65) keeps the
    fused softmax-denominator row.
  - V projection in fp8 DoubleRow too (x8/wv8 pre-interleaved on host).
  - Flat (qb, hp, kc) stream with scores emitted one step ahead of PV so
    the PE never drains at head-pair/query-block boundaries; KT/V/QT
    projections and the output projection are interleaved under the exp
    stream. Single rearranged-AP DMAs per logical tensor.
"""

import os
import sys
import numpy as np

if "/opt/trn_rl_repo" not in sys.path:
    sys.path.insert(0, "/opt/trn_rl_repo")

import ml_dtypes  # noqa: E402
import concourse.bass as bass  # noqa: E402
import concourse.mybir as mybir  # noqa: E402
from concourse import tile  # noqa: E402
from concourse import bacc  # noqa: E402
from concourse.bass_utils import run_bass_kernel_spmd  # noqa: E402
from contextlib import ExitStack  # noqa: E402

F32 = mybir.dt.float32
BF16 = mybir.dt.bfloat16
FP8 = mybir.dt.float8e4
PM = mybir.MatmulPerfMode
AF = mybir.ActivationFunctionType

N, S, D = 2, 2048, 1024
H, HD = 16, 64
NCORES = 8
CORES_PER_BATCH = 4
HPC = H // CORES_PER_BATCH      # 4 heads per core
DPC = HPC * HD                  # 256 proj cols per core
NQ = 512                        # query block
NQB = S // NQ                   # 4 query blocks
KC = S // 128                   # 16 key chunks
DC = D // 128                   # 8 contraction chunks of embed dim
NHP = HPC // 2                  # 2 head pairs per core

IO_DT = BF16
IO_NP = ml_dtypes.bfloat16
F8_NP = ml_dtypes.float8_e4m3
MODE = "bf16v6"


def build_nc(with_bias: bool = True) -> bass.Bass:
    nc = bacc.Bacc()
    xT = nc.dram_tensor("xT", [D, S], IO_DT, kind="ExternalInput")
    maskT = nc.dram_tensor("maskT", [S, S], BF16, kind="ExternalInput")
    wq = nc.dram_tensor("wq", [D, DPC], IO_DT, kind="ExternalInput")
    wk = nc.dram_tensor("wk", [D, DPC], IO_DT, kind="ExternalInput")
    wv = nc.dram_tensor("wv", [D, DPC], IO_DT, kind="ExternalInput")
    wo = nc.dram_tensor("wo", [DPC, D], IO_DT, kind="ExternalInput")
    if with_bias:
        bq = nc.dram_tensor("bq", [1, DPC], IO_DT, kind="ExternalInput")
        bk = nc.dram_tensor("bk", [1, DPC], IO_DT, kind="ExternalInput")
        bv = nc.dram_tensor("bv", [1, DPC], IO_DT, kind="ExternalInput")
    out = nc.dram_tensor("out", [D, S], F32, kind="ExternalOutput")

    scale = 1.0 / np.sqrt(HD)
    # chunk-major DRAM views for single-shot DMAs.
    # x8/wv8 are viewed with the contraction pair index innermost-second:
    # element (256*P + 128*j + p) -> [p, P, j, :], matching DoubleRow's
    # [Ki, Ko=2, free] operand layout on both sides of the V projection.
    xT_v = xT.rearrange("(i p) s -> p i s", p=128)
    wv_v = wv.rearrange("(i p) d -> p i d", p=128)
    maskT_v = maskT.rearrange("(kc p) q -> p kc q", p=128)
    wq_v = wq.rearrange("(i p) d -> p i d", p=128)
    wk_v = wk.rearrange("(i p) d -> p i d", p=128)
    wo_v = wo.rearrange("(m p) d -> p m d", p=128)
    out_v = out.rearrange("(t p) q -> p t q", p=128)

    with tile.TileContext(nc) as tc, ExitStack() as ctx:
        consts = ctx.enter_context(tc.tile_pool(name="consts", bufs=1))
        qkv_pool = ctx.enter_context(tc.tile_pool(name="qkv", bufs=1))
        proj_pool = ctx.enter_context(tc.tile_pool(name="proj", bufs=1))
        mask_pool = ctx.enter_context(tc.tile_pool(name="mask", bufs=2))
        exp_pool = ctx.enter_context(tc.tile_pool(name="exp", bufs=6))
        small = ctx.enter_context(tc.tile_pool(name="small", bufs=3))
        ost_pool = ctx.enter_context(tc.tile_pool(name="ost", bufs=1))
        att_ps = ctx.enter_context(
            tc.tile_pool(name="attps", bufs=1, space="PSUM"))

        # ---- input DMAs ----
        # DMA order = first-use order: x first half -> wq/wk (QT0/KT
        # projections can start) -> wv -> x second half -> mask0 -> wo
        xt = proj_pool.tile([128, DC, S], IO_DT, tag="xt", name="xt")
        nc.sync.dma_start(xt[:, 0:DC // 2, :], xT_v[:, 0:DC // 2, :])
        w_sb = {}
        for wname in ("wq", "wk", "wv"):
            w_sb[wname] = proj_pool.tile([128, DC, DPC], IO_DT, tag=wname,
                                         name=f"{wname}_sb")
        nc.sync.dma_start(w_sb["wq"][:], wq_v[:])
        nc.sync.dma_start(w_sb["wk"][:], wk_v[:])
        nc.sync.dma_start(w_sb["wv"][:], wv_v[:])
        nc.sync.dma_start(xt[:, DC // 2:DC, :], xT_v[:, DC // 2:DC, :])

        mt_tiles = {}

        def load_mask(qb):
            t = mask_pool.tile([128, KC, 2, NQ], BF16, tag="mt",
                               name=f"mt{qb}")
            for j in range(2):
                nc.sync.dma_start(
                    t[:, :, j, :],
                    maskT_v[:, :, NQ * qb:NQ * (qb + 1)])
            mt_tiles[qb] = t

        load_mask(0)
        WO = qkv_pool.tile([128, 2, D], IO_DT, tag="wo", name="WO")
        nc.sync.dma_start(WO[:], wo_v[:])

        # rank-1 broadcast lhsT at base partition 64 (matches pv sums row)
        ones_bc = consts.tile([HD + 1, HD], F32, tag="ones_bc")
        nc.vector.memset(ones_bc[:], 1.0)
        ones4 = consts.tile([128, NHP, 2, 1], F32, tag="ones4")
        nc.vector.memset(ones4[:], 1.0)
        if with_bias:
            ones_row = consts.tile([1, S], IO_DT, tag="ones_row")
            nc.vector.memset(ones_row[:], 1.0)

        QT = [qkv_pool.tile([128, S], IO_DT, tag=f"qt{p}", name=f"qt{p}")
              for p in range(NHP)]
        KT = [qkv_pool.tile([128, S], IO_DT, tag=f"kt{p}", name=f"kt{p}")
              for p in range(NHP)]
        V = [qkv_pool.tile([128, NHP, 2, HD + 1], IO_DT, tag=f"v{t}",
                           name=f"v{t}") for t in range(KC)]
        HO = [qkv_pool.tile([128, S], IO_DT, tag=f"ho{m}", name=f"ho{m}")
              for m in range(2)]

        b_sb = {}
        if with_bias:
            for bname, bdram in (("bq", bq), ("bk", bk), ("bv", bv)):
                b_sb[bname] = consts.tile([1, DPC], IO_DT, tag=bname,
                                          name=f"{bname}_sb")
                nc.sync.dma_start(b_sb[bname][:], bdram[:])

        def proj_qk(dst, wname, bname, m, c0):
            # dst[m][:, c0:c0+1024] in two 512 blocks, weight-stationary
            pa = att_ps.tile([128, NQ], F32, tag="ops", bufs=2, name="pa")
            pb = att_ps.tile([128, NQ], F32, tag="ops", bufs=2, name="pb")
            sa = slice(c0, c0 + NQ)
            sb_ = slice(c0 + NQ, c0 + 2 * NQ)
            last = not with_bias
            for i in range(DC):
                w_ap = w_sb[wname][:, i, 128 * m:128 * (m + 1)]
                nc.tensor.matmul(pa[:], w_ap, xt[:, i, sa],
                                 start=(i == 0),
                                 stop=(last and i == DC - 1))
                nc.tensor.matmul(pb[:], w_ap, xt[:, i, sb_],
                                 start=(i == 0),
                                 stop=(last and i == DC - 1))
            if with_bias:
                b_ap = b_sb[bname][:, 128 * m:128 * (m + 1)]
                nc.tensor.matmul(pa[:], b_ap, ones_row[:, sa],
                                 start=False, stop=True)
                nc.tensor.matmul(pb[:], b_ap, ones_row[:, sb_],
                                 start=False, stop=True)
            nc.vector.tensor_copy(dst[m][:, sa], pa[:])
            nc.vector.tensor_copy(dst[m][:, sb_], pb[:])

        def proj_v(t):
            # V natural: out[tok, d'] = x_chunk^T(as lhsT) @ Wv-chunk
            ps = att_ps.tile([128, NQ], F32, tag="ops", bufs=2, name="vps")
            for i in range(DC):
                nc.tensor.matmul(
                    ps[:, 0:DPC], xt[:, i, 128 * t:128 * (t + 1)],
                    w_sb["wv"][:, i, :],
                    start=(i == 0), stop=(not with_bias and i == DC - 1))
            if with_bias:
                nc.tensor.matmul(
                    ps[:, 0:DPC], ones_row[:, 128 * t:128 * (t + 1)],
                    b_sb["bv"][:], start=False, stop=True)
            nc.vector.tensor_copy(
                V[t][:, :, :, 0:HD],
                ps[:, 0:DPC].rearrange("p (hp h d) -> p hp h d",
                                       hp=NHP, d=HD))
            nc.vector.tensor_copy(V[t][:, :, :, HD:HD + 1], ones4[:])

        def outproj(qb, dt_):
            ps = att_ps.tile([128, NQ], F32, tag="ops", bufs=2, name="wops")
            q_sl = slice(NQ * qb, NQ * (qb + 1))
            for m in range(2):
                nc.tensor.matmul(
                    ps[:], WO[:, m, 128 * dt_:128 * (dt_ + 1)],
                    HO[m][:, q_sl], start=(m == 0), stop=(m == 1))
            nc.vector.tensor_copy(ost_all[:, dt_, :], ps[:])

        ost_all = ost_pool.tile([128, DC, NQ], F32, tag="ost",
                                name="ost_all")

        # QT for qb0 up front (both head pairs, 2x512 blocks each)
        for m in range(NHP):
            proj_qk(QT, "wq", "bq", m, 0)

        pvs = {}
        ex_tiles = {}

        def emit_jit(qb, hp, kc):
            if hp == 0 and kc == 0 and qb + 1 < NQB:
                load_mask(qb + 1)
            if qb == 0 and hp == 0:
                # KT in 1024-col weight-stationary units; V fp8 per chunk
                if kc in (0, 4, 8, 12):
                    km, kc0 = {0: (0, 0), 4: (1, 0),
                               8: (0, 1024), 12: (1, 1024)}[kc]
                    proj_qk(KT, "wk", "bk", km, kc0)
                proj_v(kc)
            if hp == 0 and qb == 1 and kc in (11, 13):
                # QT cols [1024,2048) (qb2+qb3); [0,1024) was done up front
                proj_qk(QT, "wq", "bq", (kc - 11) // 2, 1024)
            if hp == 0 and qb > 0 and 2 <= kc < 10:
                outproj(qb - 1, kc - 2)
            if hp == 1 and kc == 0 and qb > 0:
                nc.sync.dma_start(
                    out_v[:, :, NQ * (qb - 1):NQ * qb], ost_all[:])

        def emit_sc(qb, hp, kc):
            q_sl = slice(NQ * qb, NQ * (qb + 1))
            k_sl = slice(128 * kc, 128 * (kc + 1))
            sc = att_ps.tile([128, 2 * NQ], F32, tag="sc", bufs=2,
                             name="sc")
            for j in range(2):
                nc.tensor.matmul(
                    sc[:, NQ * j:NQ * (j + 1)],
                    KT[hp][64 * j:64 * (j + 1), k_sl],
                    QT[hp][64 * j:64 * (j + 1), q_sl],
                    start=True, stop=True)
            ex = exp_pool.tile([128, 2 * NQ], IO_DT, tag="ex", name="ex")
            ex_tiles[(qb, hp, kc)] = ex
            nc.scalar.activation(ex[:], sc[:], AF.Exp, scale=scale)
            mt_ap = mt_tiles[qb][:, kc, :, :].rearrange("p a b -> p (a b)")
            nc.vector.tensor_mul(ex[:], ex[:], mt_ap)

        def emit_pv(qb, hp, kc):
            if kc == 0:
                pvs[(qb, hp)] = [
                    att_ps.tile([HD + 1, NQ], F32, tag=f"pv{j}", bufs=1,
                                name=f"pv{j}") for j in range(2)]
            pv = pvs[(qb, hp)]
            ex = ex_tiles.pop((qb, hp, kc))
            for j in range(2):
                nc.tensor.matmul(
                    pv[j][:],
                    V[kc][:, hp, j, :],
                    ex[:, NQ * j:NQ * (j + 1)],
                    start=(kc == 0), stop=(kc == KC - 1))

        def emit_epilogue(qb, hp):
            q_sl = slice(NQ * qb, NQ * (qb + 1))
            pv = pvs.pop((qb, hp))
            srows = []
            for j in range(2):
                srow = small.tile([HD + 1, NQ], F32, tag="srow",
                                  name="srow")
                nc.vector.tensor_copy(srow[HD:HD + 1, :],
                                      pv[j][HD:HD + 1, :])
                srows.append(srow)
            for j in range(2):
                srow = srows[j]
                bps = att_ps.tile([128, NQ], F32, tag="ops", bufs=2,
                                  name="bps")
                nc.tensor.matmul(bps[0:HD, :], ones_bc[HD:HD + 1, :],
                                 srow[HD:HD + 1, :], start=True, stop=True)
                bc = small.tile([HD, NQ], F32, tag="bc", name="bc")
                nc.vector.reciprocal_approx_fast(bc[:], bps[0:HD, :])
                if j == 0:
                    nc.vector.tensor_mul(HO[hp][0:HD, q_sl],
                                         pv[j][0:HD, :], bc[:])
                else:
                    ho_t = small.tile([HD, NQ], IO_DT, tag="hot",
                                      name="hot")
                    nc.vector.tensor_mul(ho_t[:], pv[j][0:HD, :], bc[:])
                    nc.sync.dma_start(HO[hp][HD:128, q_sl], ho_t[:])

        steps = [(qb, hp, kc)
                 for qb in range(NQB)
                 for hp in range(NHP)
                 for kc in range(KC)]
        LAG = 1
        pending = []
        for st in steps:
            emit_jit(*st)
            emit_sc(*st)
            pending.append(st)
            if len(pending) > LAG:
                p = pending.pop(0)
                emit_pv(*p)
                if p[2] == KC - 1:
                    emit_epilogue(p[0], p[1])
        for p in pending:
            emit_pv(*p)
            if p[2] == KC - 1:
                emit_epilogue(p[0], p[1])

        # tail: output projection for the last query block
        for dt_ in range(DC):
            outproj(NQB - 1, dt_)
        nc.sync.dma_start(out_v[:, :, NQ * (NQB - 1):S], ost_all[:])
    nc.finalize()
    return nc


def shard_inputs(x, mask, Wq, bq, Wk, bk, Wv, bv, Wo, bo):
    x = np.asarray(x, dtype=np.float32)
    mask = np.asarray(mask)
    xT = [np.ascontiguousarray(x[n].T) for n in range(N)]
    maskT = [np.ascontiguousarray(mask[n, 0].T).astype(IO_NP)
             for n in range(N)]
    in_maps = []
    for c in range(NCORES):
        n = c // CORES_PER_BATCH
        lo = (c % CORES_PER_BATCH) * DPC
        hi = lo + DPC
        in_maps.append({
            "xT": xT[n].astype(IO_NP),
            "maskT": maskT[n],
            "wq": np.ascontiguousarray(np.asarray(Wq)[:, lo:hi]).astype(IO_NP),
            "wk": np.ascontiguousarray(np.asarray(Wk)[:, lo:hi]).astype(IO_NP),
            "wv": np.ascontiguousarray(np.asarray(Wv)[:, lo:hi]).astype(IO_NP),
            "wo": np.ascontiguousarray(np.asarray(Wo)[lo:hi, :]).astype(IO_NP),
            "bq": np.asarray(bq, dtype=np.float32)[lo:hi].reshape(1, DPC).astype(IO_NP),
            "bk": np.asarray(bk, dtype=np.float32)[lo:hi].reshape(1, DPC).astype(IO_NP),
            "bv": np.asarray(bv, dtype=np.float32)[lo:hi].reshape(1, DPC).astype(IO_NP),
        })
    return in_maps


LAST_RESULTS = None


def kernel(x, mask, Wq, bq, Wk, bk, Wv, bv, Wo, bo):
    global LAST_RESULTS
    with_bias = any(np.any(np.asarray(b)) for b in (bq, bk, bv))
    nc = build_nc(with_bias=with_bias)
    in_maps = shard_inputs(x, mask, Wq, bq, Wk, bk, Wv, bv, Wo, bo)
    if not with_bias:
        for im in in_maps:
            im.pop("bq"), im.pop("bk"), im.pop("bv")
    trace = bool(os.environ.get("ATT_TRACE"))
    res = run_bass_kernel_spmd(nc, in_maps, list(range(NCORES)), trace=trace)
    LAST_RESULTS = res
    outs = [np.asarray(r["out"], dtype=np.float32) for r in res.results]
    y = np.empty((N, S, D), dtype=np.float32)
    bo_f = np.asarray(bo, dtype=np.float32)
    for n in range(N):
        acc = outs[n * CORES_PER_BATCH]
        for c in range(1, CORES_PER_BATCH):
            acc = acc + outs[n * CORES_PER_BATCH + c]
        y[n] = acc.T + bo_f
    return y
